# revision 7
# baseline (speedup 1.0000x reference)
"""Location-sensitive attention kernel for 8 Trainium2 NeuronCores.

Strategy: data-parallel over batch (32 rows -> 4 per core, weights replicated).

Per batch row b (T=4096, D=512), computed on device:
  scores.T[d,t] accum = sum_k Wm[k,d] * memT[k,t]          (4 f32r matmuls, K=128)
                      + sum_r Wcomb[r,d] * X[r,t]          (1 f32r matmul, K=62)
    where Wcomb = im2col(Wconv) @ Wloc is a host-side weight-only constant fold
    and X is the im2col of [prev_attn; cum_attn] (pure data relabeling).
  tanh fused on ACT: th = tanh(psum + bias_b[d]) with per-partition bias
    bias_b = query@Wq + bq + bm + bloc + bconv@Wloc  (q-proj on device)
  e[t]   = sum_d wv[d] * th[d,t]    (fp16 matmuls, M=1)
  masked softmax over t (additive mask; exp fused with row-sum accumulation)
  ctx[d] = sum_t a[t] * mem[t,d]    (bf16 matmuls, a as stationary operand)
"""

import numpy as np
import ml_dtypes

B, T, D = 32, 4096, 512
KW, C = 31, 32
NCORES = 8
BPC = B // NCORES  # batch rows per core
KC = D // 128      # 4 contraction chunks
DO = D // 128      # 4 output d chunks
TJ = T // 512      # 8 t chunks of 512
LPAD = 15 + T + 17  # padded attn length (window max index 30+4095 < 4128)

_CACHE = {}


def _build():
    import concourse.bacc as bacc
    import concourse.mybir as mybir
    from concourse.tile import TileContext
    from contextlib import ExitStack

    f32 = mybir.dt.float32
    f32r = mybir.dt.float32r
    f16 = mybir.dt.float16
    bf16 = mybir.dt.bfloat16
    AF = mybir.ActivationFunctionType
    OP = mybir.AluOpType

    nc = bacc.Bacc("TRN2", target_bir_lowering=False, debug=False,
                   num_devices=NCORES)

    memT_d = nc.dram_tensor("memT", [BPC, D, T], f32r, kind="ExternalInput").ap()
    mem16_d = nc.dram_tensor("mem16", [BPC, T, D], bf16, kind="ExternalInput").ap()
    x_d = nc.dram_tensor("xim", [BPC, 62, T], f32r, kind="ExternalInput").ap()
    qT_d = nc.dram_tensor("qT", [D, BPC], f32r, kind="ExternalInput").ap()
    wm_d = nc.dram_tensor("wm", [D, D], f32r, kind="ExternalInput").ap()
    wq_d = nc.dram_tensor("wq", [D, D], f32r, kind="ExternalInput").ap()
    wcomb_d = nc.dram_tensor("wcomb", [62, D], f32r, kind="ExternalInput").ap()
    wv_d = nc.dram_tensor("wv16", [128, DO], f16, kind="ExternalInput").ap()
    bconst_d = nc.dram_tensor("bconst", [128, DO], f32, kind="ExternalInput").ap()
    madd_d = nc.dram_tensor("madd", [BPC, T], f32, kind="ExternalInput").ap()

    ctx_d = nc.dram_tensor("ctx", [BPC, D], f32, kind="ExternalOutput").ap()
    a_d = nc.dram_tensor("attn", [BPC, T], f32, kind="ExternalOutput").ap()

    with ExitStack() as st:
        tc = st.enter_context(TileContext(nc))
        wpool = st.enter_context(tc.tile_pool(name="w", bufs=1))
        mtpool = st.enter_context(tc.tile_pool(name="mt", bufs=10))
        xpool = st.enter_context(tc.tile_pool(name="xi", bufs=4))
        thpool = st.enter_context(tc.tile_pool(name="th", bufs=6))
        cnpool = st.enter_context(tc.tile_pool(name="cn", bufs=4))
        sme = st.enter_context(tc.tile_pool(name="sme", bufs=2))
        sm1 = st.enter_context(tc.tile_pool(name="sm1", bufs=2))
        psS = st.enter_context(tc.tile_pool(name="psS", bufs=2, space="PSUM"))
        psE = st.enter_context(tc.tile_pool(name="psE", bufs=2, space="PSUM"))
        psC = st.enter_context(tc.tile_pool(name="psC", bufs=2, space="PSUM"))
        drpool = st.enter_context(tc.tile_pool(name="dr", bufs=2, space="DRAM"))

        # ---- weights / constants (loaded once) ----
        wm_sb = []
        for k in range(KC):
            t = wpool.tile([128, D], f32r, tag=f"wm{k}")
            nc.sync.dma_start(out=t[:], in_=wm_d[k * 128:(k + 1) * 128, :])
            wm_sb.append(t)
        wcomb_sb = wpool.tile([62, D], f32r, tag="wcomb")
        nc.sync.dma_start(out=wcomb_sb[:], in_=wcomb_d[:])
        wv_sb = wpool.tile([128, DO], f16, tag="wv")
        nc.sync.dma_start(out=wv_sb[:], in_=wv_d[:])
        bias_sb = wpool.tile([128, DO * BPC], f32, tag="bias")

        # bias_bd[do] = sum_k Wq[k,do].T @ qT[k] + bconst[do]  -> [128, BPC]
        with tc.tile_pool(name="qtmp", bufs=1) as qpool, \
             tc.tile_pool(name="psQ", bufs=1, space="PSUM") as psQ:
            bconst_sb = qpool.tile([128, DO], f32, tag="bconst")
            nc.sync.dma_start(out=bconst_sb[:], in_=bconst_d[:])
            qT_sb = qpool.tile([128, KC * BPC], f32r, tag="qT")
            for k in range(KC):
                nc.sync.dma_start(out=qT_sb[:, k * BPC:(k + 1) * BPC],
                                  in_=qT_d[k * 128:(k + 1) * 128, :])
            wq_sb = []
            for k in range(KC):
                t2 = qpool.tile([128, D], f32r, tag=f"wq{k}")
                nc.sync.dma_start(out=t2[:], in_=wq_d[k * 128:(k + 1) * 128, :])
                wq_sb.append(t2)
            for do in range(DO):
                pq = psQ.tile([128, BPC], f32)
                for k in range(KC):
                    nc.tensor.matmul(pq[:], wq_sb[k][:, do * 128:(do + 1) * 128],
                                     qT_sb[:, k * BPC:(k + 1) * BPC],
                                     start=(k == 0), stop=(k == KC - 1))
                nc.vector.tensor_scalar(
                    out=bias_sb[:, do * BPC:(do + 1) * BPC], in0=pq[:],
                    scalar1=bconst_sb[:, do:do + 1], scalar2=None, op0=OP.add)

        for b in range(BPC):
            # ---- score pass for row b ----
            mt = [[None] * KC for _ in range(4)]
            for q in range(4):  # quarters of T, then k chunks (alloc order!)
                for k in range(KC):
                    m = mtpool.tile([128, 1024], f32r, tag="mt")
                    nc.sync.dma_start(
                        out=m[:],
                        in_=memT_d[b, k * 128:(k + 1) * 128,
                                   q * 1024:(q + 1) * 1024])
                    mt[q][k] = m
            e_b = sme.tile([1, T], f32, tag="e")
            ma_b = sm1.tile([1, T], f32, tag="ma")
            nc.sync.dma_start(out=ma_b[:], in_=madd_d[b:b + 1, :])
            for tj in range(TJ):
                xt = xpool.tile([62, 512], f32r, tag="xt")
                nc.sync.dma_start(out=xt[:],
                                  in_=x_d[b, :, tj * 512:(tj + 1) * 512])
                pe_ = psE.tile([1, 512], f32)
                for do in range(DO):
                    ps = psS.tile([128, 512], f32)
                    for k in range(KC):
                        src = mt[tj // 2][k]
                        nc.tensor.matmul(
                            ps[:], wm_sb[k][:, do * 128:(do + 1) * 128],
                            src[:, (tj % 2) * 512:(tj % 2 + 1) * 512],
                            start=(k == 0), stop=False)
                    nc.tensor.matmul(ps[:],
                                     wcomb_sb[:, do * 128:(do + 1) * 128],
                                     xt[:], start=False, stop=True)
                    th = thpool.tile([128, 512], f16, tag="th")
                    nc.scalar.activation(
                        th[:], ps[:], AF.Tanh,
                        bias=bias_sb[:, do * BPC + b:do * BPC + b + 1])
                    nc.tensor.matmul(pe_[:], wv_sb[:, do:do + 1], th[:],
                                     start=(do == 0), stop=(do == DO - 1))
                # e segment = wv-reduced scores + additive mask
                nc.vector.tensor_add(e_b[:, tj * 512:(tj + 1) * 512], pe_[:],
                                     ma_b[:, tj * 512:(tj + 1) * 512])

            # ---- softmax for row b (no max-sub needed: |e|<=12, mask->-1e9) --
            xs = sme.tile([1, T], f32, tag="e")  # shares slots with e_b
            se = sm1.tile([1, 1], f32, tag="se")
            nc.scalar.activation(xs[:], e_b[:], AF.Exp, accum_out=se[:])
            ri = sm1.tile([1, 1], f32, tag="ri")
            nc.vector.reciprocal(ri[:], se[:])
            a_b = sm1.tile([1, T], f32, tag="a")
            nc.vector.tensor_scalar(out=a_b[:], in0=xs[:], scalar1=ri[:],
                                    scalar2=None, op0=OP.mult)
            nc.sync.dma_start(out=a_d[b:b + 1, :], in_=a_b[:])
            a16 = sm1.tile([1, T], bf16, tag="a16")
            nc.vector.tensor_copy(a16[:], a_b[:])
            ascr = drpool.tile([1, T], bf16, tag="ascr")
            nc.sync.dma_start(out=ascr[:], in_=a16[:])

            # ---- ctx pass for row b: ctx[d] = sum_t a[t] mem[t,d] ----
            aT = sm1.tile([128, 32], bf16, tag="aT")
            nc.sync.dma_start(out=aT[:],
                              in_=ascr[0].rearrange("(c p) -> p c", p=128))
            pc = psC.tile([1, 512], f32)
            for tt in range(32):
                if tt % 4 == 0:
                    cn = cnpool.tile([128, 4, 512], bf16, tag="cn")
                    nc.sync.dma_start(
                        out=cn[:],
                        in_=mem16_d[b, tt * 128:(tt + 4) * 128, :]
                        .rearrange("(j p) d -> p j d", p=128))
                nc.tensor.matmul(pc[:], aT[:, tt:tt + 1], cn[:, tt % 4, :],
                                 start=(tt == 0), stop=(tt == 31))
            cst = sm1.tile([1, D], f32, tag="cst")
            nc.scalar.copy(cst[:], pc[:])
            nc.sync.dma_start(out=ctx_d[b:b + 1, :], in_=cst[:])

    nc.compile()
    return nc


def _prep(query, memory, prev_attn, cum_attn, mask,
          Wq, bq, Wm, bm, Wconv, bconv, Wloc, bloc, wv, bv):
    """Host-side input marshaling + weight-only constant folding."""
    f32 = np.float32
    query = np.asarray(query, f32)
    memory = np.ascontiguousarray(np.asarray(memory, f32))
    prev_attn = np.asarray(prev_attn, f32)
    cum_attn = np.asarray(cum_attn, f32)
    mask = np.asarray(mask)

    memT = np.ascontiguousarray(memory.transpose(0, 2, 1))          # [B, D, T]
    mem16 = memory.astype(ml_dtypes.bfloat16)                       # [B, T, D]

    # im2col of location features (zero FLOPs, data relabeling)
    ap = np.zeros((B, 2, LPAD), f32)
    ap[:, 0, 15:15 + T] = prev_attn
    ap[:, 1, 15:15 + T] = cum_attn
    sw = np.lib.stride_tricks.sliding_window_view(ap, T, axis=2)    # [B,2,33,T]
    xim = np.ascontiguousarray(
        sw[:, :, :KW, :].reshape(B, 2 * KW, T))                      # [B, 62, T]

    # weight-only constant folds
    Wc2 = np.asarray(Wconv, f32).transpose(1, 2, 0).reshape(2 * KW, C)
    Wcomb = np.ascontiguousarray(Wc2 @ np.asarray(Wloc, f32))        # [62, D]
    bconst = (np.asarray(bq, f32) + np.asarray(bm, f32)
              + np.asarray(bloc, f32)
              + np.asarray(bconv, f32) @ np.asarray(Wloc, f32))      # [D]
    bconst_t = np.ascontiguousarray(bconst.reshape(DO, 128).T)       # [128, DO]
    wv16 = np.ascontiguousarray(
        np.asarray(wv, f32).reshape(DO, 128).T).astype(np.float16)   # [128, DO]
    qT = np.ascontiguousarray(query.T)                               # [D, B]

    # additive mask: e + (-1e9) underflows exp to exactly 0, matching the
    # reference's where(mask, -1e9, e).  Fully-masked rows keep position 0
    # unmasked -> softmax = exact one-hot at 0 (matches the e[:,0]=0 rule).
    mf = mask.astype(f32)
    madd = np.float32(-1e9) * mf
    madd[mask.all(axis=1), 0] = 0.0

    wm = np.ascontiguousarray(np.asarray(Wm, f32))
    wq = np.ascontiguousarray(np.asarray(Wq, f32))

    in_maps = []
    for c in range(NCORES):
        s = slice(c * BPC, (c + 1) * BPC)
        in_maps.append({
            "memT": memT[s], "mem16": mem16[s], "xim": xim[s],
            "qT": np.ascontiguousarray(qT[:, s]),
            "wm": wm, "wq": wq, "wcomb": Wcomb, "wv16": wv16,
            "bconst": bconst_t,
            "madd": np.ascontiguousarray(madd[s]),
        })
    return in_maps


def _run(**inputs):
    from concourse.bass_utils import run_bass_kernel_spmd
    if "nc" not in _CACHE:
        _CACHE["nc"] = _build()
    nc = _CACHE["nc"]
    in_maps = _prep(**inputs)
    res = run_bass_kernel_spmd(nc, in_maps, list(range(NCORES)))
    ctx = np.concatenate([res.results[c]["ctx"] for c in range(NCORES)], axis=0)
    a = np.concatenate([res.results[c]["attn"] for c in range(NCORES)], axis=0)
    return ctx.astype(np.float32), a.astype(np.float32), res


def kernel(**inputs):
    ctx, a, _ = _run(**inputs)
    return ctx, a


# revision 18
# speedup vs baseline: 1.2556x; 1.2556x over previous
"""Location-sensitive attention kernel for 8 Trainium2 NeuronCores.

Strategy: data-parallel over batch (32 rows -> 4 per core, weights replicated).

Per batch row b (T=4096, D=512), computed on device:
  scores.T[d,t] accum = sum_k Wm[k,d] * memT[k,t]          (4 f32r matmuls, K=128)
                      + sum_r Wcomb[r,d] * X[r,t]          (1 f32r matmul, K=62)
    where Wcomb = im2col(Wconv) @ Wloc is a host-side weight-only constant fold
    and X is the im2col of [prev_attn; cum_attn] (pure data relabeling).
  tanh fused on ACT: th = tanh(psum + bias_b[d]) with per-partition bias
    bias_b = query@Wq + bq + bm + bloc + bconv@Wloc  (q-proj on device)
  e[t]   = sum_d wv[d] * th[d,t]    (fp16 matmuls, M=1)
  masked softmax over t (additive mask; exp fused with row-sum accumulation)
  ctx[d] = sum_t a[t] * mem[t,d]    (f32r matmuls, a as stationary operand)
"""

import numpy as np

B, T, D = 32, 4096, 512
KW, C = 31, 32
NCORES = 8
BPC = B // NCORES  # batch rows per core
KC = D // 128      # 4 contraction chunks
DO = D // 128      # 4 output d chunks
TJ = T // 512      # 8 t chunks of 512
LPAD = 15 + T + 17  # padded attn length (window max index 30+4095 < 4128)

_CACHE = {}


def _build(reps=1):
    import concourse.bacc as bacc
    import concourse.mybir as mybir
    from concourse.tile import TileContext
    from contextlib import ExitStack

    f32 = mybir.dt.float32
    f32r = mybir.dt.float32r
    f16 = mybir.dt.float16
    AF = mybir.ActivationFunctionType
    OP = mybir.AluOpType

    nc = bacc.Bacc("TRN2", target_bir_lowering=False, debug=False,
                   num_devices=NCORES)

    memT_d = nc.dram_tensor("memT", [BPC, D, T], f32r, kind="ExternalInput").ap()
    memn_d = nc.dram_tensor("memn", [BPC, T, D], f32r, kind="ExternalInput").ap()
    x_d = nc.dram_tensor("xim", [BPC, 62, T], f32r, kind="ExternalInput").ap()
    qT_d = nc.dram_tensor("qT", [D, BPC], f32r, kind="ExternalInput").ap()
    wm_d = nc.dram_tensor("wm", [D, D], f32r, kind="ExternalInput").ap()
    wq_d = nc.dram_tensor("wq", [D, D], f32r, kind="ExternalInput").ap()
    wcomb_d = nc.dram_tensor("wcomb", [62, D], f32r, kind="ExternalInput").ap()
    wv_d = nc.dram_tensor("wv16", [128, DO], f16, kind="ExternalInput").ap()
    bconst_d = nc.dram_tensor("bconst", [128, DO], f32, kind="ExternalInput").ap()
    madd_d = nc.dram_tensor("madd", [BPC, T], f32, kind="ExternalInput").ap()

    ctx_d = nc.dram_tensor("ctx", [BPC, D], f32, kind="ExternalOutput").ap()
    a_d = nc.dram_tensor("attn", [BPC, T], f32, kind="ExternalOutput").ap()

    with ExitStack() as st:
        tc = st.enter_context(TileContext(nc))
        wpool = st.enter_context(tc.tile_pool(name="w", bufs=1))
        mtpool = st.enter_context(tc.tile_pool(name="mt", bufs=10))
        xpool = st.enter_context(tc.tile_pool(name="xi", bufs=4))
        thpool = st.enter_context(tc.tile_pool(name="th", bufs=6))
        cnpool = st.enter_context(tc.tile_pool(name="cn", bufs=4))
        sme = st.enter_context(tc.tile_pool(name="sme", bufs=2))
        sm1 = st.enter_context(tc.tile_pool(name="sm1", bufs=2))
        sma = st.enter_context(tc.tile_pool(name="sma", bufs=1))
        psS = st.enter_context(tc.tile_pool(name="psS", bufs=2, space="PSUM"))
        psE = st.enter_context(tc.tile_pool(name="psE", bufs=2, space="PSUM"))
        psC = st.enter_context(tc.tile_pool(name="psC", bufs=2, space="PSUM"))
        drpool = st.enter_context(tc.tile_pool(name="dr", bufs=2, space="DRAM"))

        # ---- weights / constants (loaded once) ----
        wm_sb = []
        for k in range(KC):
            t = wpool.tile([128, D], f32r, tag=f"wm{k}")
            nc.sync.dma_start(out=t[:], in_=wm_d[k * 128:(k + 1) * 128, :])
            wm_sb.append(t)
        wcomb_sb = wpool.tile([62, D], f32r, tag="wcomb")
        nc.sync.dma_start(out=wcomb_sb[:], in_=wcomb_d[:])
        wv_sb = wpool.tile([128, DO], f16, tag="wv")
        nc.sync.dma_start(out=wv_sb[:], in_=wv_d[:])
        bias_sb = wpool.tile([128, DO * BPC], f32, tag="bias")

        # bias_bd[do] = sum_k Wq[k,do].T @ qT[k] + bconst[do]  -> [128, BPC]
        with tc.tile_pool(name="qtmp", bufs=1) as qpool, \
             tc.tile_pool(name="psQ", bufs=1, space="PSUM") as psQ:
            bconst_sb = qpool.tile([128, DO], f32, tag="bconst")
            nc.sync.dma_start(out=bconst_sb[:], in_=bconst_d[:])
            qT_sb = qpool.tile([128, KC * BPC], f32r, tag="qT")
            for k in range(KC):
                nc.sync.dma_start(out=qT_sb[:, k * BPC:(k + 1) * BPC],
                                  in_=qT_d[k * 128:(k + 1) * 128, :])
            wq_sb = []
            for k in range(KC):
                t2 = qpool.tile([128, D], f32r, tag=f"wq{k}")
                nc.sync.dma_start(out=t2[:], in_=wq_d[k * 128:(k + 1) * 128, :])
                wq_sb.append(t2)
            for do in range(DO):
                pq = psQ.tile([128, BPC], f32)
                for k in range(KC):
                    nc.tensor.matmul(pq[:], wq_sb[k][:, do * 128:(do + 1) * 128],
                                     qT_sb[:, k * BPC:(k + 1) * BPC],
                                     start=(k == 0), stop=(k == KC - 1))
                nc.vector.tensor_scalar(
                    out=bias_sb[:, do * BPC:(do + 1) * BPC], in0=pq[:],
                    scalar1=bconst_sb[:, do:do + 1], scalar2=None, op0=OP.add)

        def _row(b):
            # ---- score pass for row b ----
            mt = [[None] * KC for _ in range(4)]
            for q in range(4):  # quarters of T, then k chunks (alloc order!)
                for k in range(KC):
                    m = mtpool.tile([128, 1024], f32r, tag="mt")
                    nc.sync.dma_start(
                        out=m[:],
                        in_=memT_d[b, k * 128:(k + 1) * 128,
                                   q * 1024:(q + 1) * 1024])
                    mt[q][k] = m
            e_b = sme.tile([1, T], f32, tag="e")
            ma_b = sma.tile([1, T], f32, tag="ma")
            nc.sync.dma_start(out=ma_b[:], in_=madd_d[b:b + 1, :])
            for tj in range(TJ):
                xt = xpool.tile([62, 512], f32r, tag="xt")
                nc.sync.dma_start(out=xt[:],
                                  in_=x_d[b, :, tj * 512:(tj + 1) * 512])
                pe_ = psE.tile([1, 512], f32)
                for do in range(DO):
                    ps = psS.tile([128, 512], f32)
                    for k in range(KC):
                        src = mt[tj // 2][k]
                        nc.tensor.matmul(
                            ps[:], wm_sb[k][:, do * 128:(do + 1) * 128],
                            src[:, (tj % 2) * 512:(tj % 2 + 1) * 512],
                            start=(k == 0), stop=False)
                    nc.tensor.matmul(ps[:],
                                     wcomb_sb[:, do * 128:(do + 1) * 128],
                                     xt[:], start=False, stop=True)
                    th = thpool.tile([128, 512], f16, tag="th")
                    nc.scalar.activation(
                        th[:], ps[:], AF.Tanh,
                        bias=bias_sb[:, do * BPC + b:do * BPC + b + 1])
                    nc.tensor.matmul(pe_[:], wv_sb[:, do:do + 1], th[:],
                                     start=(do == 0), stop=(do == DO - 1))
                # e segment = wv-reduced scores + additive mask
                nc.vector.tensor_add(e_b[:, tj * 512:(tj + 1) * 512], pe_[:],
                                     ma_b[:, tj * 512:(tj + 1) * 512])

            # ---- softmax for row b (no max-sub needed: |e|<=12, mask->-1e9) --
            xs = sme.tile([1, T], f32, tag="e")  # shares slots with e_b
            se = sm1.tile([1, 1], f32, tag="se")
            nc.scalar.activation(xs[:], e_b[:], AF.Exp, accum_out=se[:])
            ri = sm1.tile([1, 1], f32, tag="ri")
            nc.vector.reciprocal(ri[:], se[:])
            a_b = sm1.tile([1, T], f32, tag="a")
            nc.vector.tensor_scalar(out=a_b[:], in0=xs[:], scalar1=ri[:],
                                    scalar2=None, op0=OP.mult)
            nc.sync.dma_start(out=a_d[b:b + 1, :], in_=a_b[:])
            ascr = drpool.tile([1, T], f32r, tag="ascr")
            nc.sync.dma_start(out=ascr[:], in_=a_b[:].bitcast(f32r))
            # transposed view of a for the ctx matmul (strided DMA read)
            aT = sm1.tile([128, 32], f32r, tag="aT")
            nc.sync.dma_start(out=aT[:],
                              in_=ascr[0].rearrange("(c p) -> p c", p=128))
            return aT

        def _ctx(b, aT):
            # ---- ctx pass for row b: ctx[d] = sum_t a[t] mem[t,d] ----
            pc = psC.tile([1, 512], f32)
            for tt in range(32):
                if tt % 4 == 0:
                    cn = cnpool.tile([128, 4, 512], f32r, tag="cn")
                    nc.sync.dma_start(
                        out=cn[:],
                        in_=memn_d[b, tt * 128:(tt + 4) * 128, :]
                        .rearrange("(j p) d -> p j d", p=128))
                nc.tensor.matmul(pc[:], aT[:, tt:tt + 1], cn[:, tt % 4, :],
                                 start=(tt == 0), stop=(tt == 31))
            cst = sm1.tile([1, D], f32, tag="cst")
            nc.scalar.copy(cst[:], pc[:])
            nc.sync.dma_start(out=ctx_d[b:b + 1, :], in_=cst[:])

        for _rep in range(reps):
            pend = None
            for b in range(BPC):
                aT = _row(b)
                if pend is not None:
                    _ctx(*pend)
                pend = (b, aT)
            _ctx(*pend)

    nc.compile()
    return nc


def _prep(query, memory, prev_attn, cum_attn, mask,
          Wq, bq, Wm, bm, Wconv, bconv, Wloc, bloc, wv, bv):
    """Host-side input marshaling + weight-only constant folding."""
    f32 = np.float32
    query = np.asarray(query, f32)
    memory = np.ascontiguousarray(np.asarray(memory, f32))
    prev_attn = np.asarray(prev_attn, f32)
    cum_attn = np.asarray(cum_attn, f32)
    mask = np.asarray(mask)

    memT = np.ascontiguousarray(memory.transpose(0, 2, 1))          # [B, D, T]

    # im2col of location features (zero FLOPs, data relabeling)
    ap = np.zeros((B, 2, LPAD), f32)
    ap[:, 0, 15:15 + T] = prev_attn
    ap[:, 1, 15:15 + T] = cum_attn
    sw = np.lib.stride_tricks.sliding_window_view(ap, T, axis=2)    # [B,2,33,T]
    xim = np.ascontiguousarray(
        sw[:, :, :KW, :].reshape(B, 2 * KW, T))                      # [B, 62, T]

    # weight-only constant folds
    Wc2 = np.asarray(Wconv, f32).transpose(1, 2, 0).reshape(2 * KW, C)
    Wcomb = np.ascontiguousarray(Wc2 @ np.asarray(Wloc, f32))        # [62, D]
    bconst = (np.asarray(bq, f32) + np.asarray(bm, f32)
              + np.asarray(bloc, f32)
              + np.asarray(bconv, f32) @ np.asarray(Wloc, f32))      # [D]
    bconst_t = np.ascontiguousarray(bconst.reshape(DO, 128).T)       # [128, DO]
    wv16 = np.ascontiguousarray(
        np.asarray(wv, f32).reshape(DO, 128).T).astype(np.float16)   # [128, DO]
    qT = np.ascontiguousarray(query.T)                               # [D, B]

    # additive mask: e + (-1e9) underflows exp to exactly 0, matching the
    # reference's where(mask, -1e9, e).  Fully-masked rows keep position 0
    # unmasked -> softmax = exact one-hot at 0 (matches the e[:,0]=0 rule).
    mf = mask.astype(f32)
    madd = np.float32(-1e9) * mf
    madd[mask.all(axis=1), 0] = 0.0

    wm = np.ascontiguousarray(np.asarray(Wm, f32))
    wq = np.ascontiguousarray(np.asarray(Wq, f32))

    in_maps = []
    for c in range(NCORES):
        s = slice(c * BPC, (c + 1) * BPC)
        in_maps.append({
            "memT": memT[s], "memn": memory[s], "xim": xim[s],
            "qT": np.ascontiguousarray(qT[:, s]),
            "wm": wm, "wq": wq, "wcomb": Wcomb, "wv16": wv16,
            "bconst": bconst_t,
            "madd": np.ascontiguousarray(madd[s]),
        })
    return in_maps


def _run(**inputs):
    from concourse.bass_utils import run_bass_kernel_spmd
    if "nc" not in _CACHE:
        _CACHE["nc"] = _build()
    nc = _CACHE["nc"]
    in_maps = _prep(**inputs)
    res = run_bass_kernel_spmd(nc, in_maps, list(range(NCORES)))
    ctx = np.concatenate([res.results[c]["ctx"] for c in range(NCORES)], axis=0)
    a = np.concatenate([res.results[c]["attn"] for c in range(NCORES)], axis=0)
    return ctx.astype(np.float32), a.astype(np.float32), res


def kernel(**inputs):
    ctx, a, _ = _run(**inputs)
    return ctx, a
